# revision 53
# baseline (speedup 1.0000x reference)
"""Trainium2 Bass kernel for GQA causal attention (B=2, L=2048, D=2048, H=16, KVH=4).

Sharding: 8 cores = 2-way data-parallel (batch) x 4-way tensor-parallel (heads).
Each core handles one batch element, 4 query heads, and their shared KV head.
Wo is row-sharded; the host sums the 4 partial outputs per batch.

Mixed-precision fp8 (e4m3) with DoubleRow matmuls, tuned against the CoreSim
cost model (DoubleRow = 0.5 cycles/row with K=256 per instruction = 4x bf16):
  - Q/K projections: 2-term hi-lo split of x (xh*w + xl*w), w plain fp8.
    Residual error = w quantization only, softmax-dampened downstream.
  - V projection: 3-term hi-lo (xh*wh + xl*wh + xh*wl) -> near-exact.
  - Scores: bf16 (full accuracy); causal diagonal via suffix-restricted
    matmuls + suffix exps + affine_select on the boundary windows (the
    attn@v/rowsum matmuls also suffix-skip the all-zero masked columns).
  - Attention weights (exp output): plain fp8 with +0.75 exp bias (folded
    into softmax, cancels exactly in the normalization).
  - rowsum + attn@v: DoubleRow over k-tile pairs; v is hi-lo (2 accumulating
    matmuls); the "ones" vector carries Sv/Sao so normalization scales land
    for free.
  - Wo: 3-term hi-lo fp8.
All fp8 tensors are pre-scaled into e4m3's normal range (x*8, W*512, v*8,
ao*16); descales fold into ACT copy scales, the exp scale, and the ones value.

Softmax normalization: the rowsum ones-matmul uses an all-ones [128,2,128]
stationary so every output partition carries the sum (matmul cost depends
only on free size), then reciprocal on DVE and one multiply - no separate
broadcast step.

Scheduling: projections stream x in hi/lo chunk pairs with 8 concurrent
psum groups consuming each chunk as it lands; psums are evicted by a single
ACT copy into persistent staging tiles (cos/sin duplicated to 128 partitions
so rope needs no rebase bounce) and the rope DVE chains for heads 1-3 are
deferred into the attention phase; Wo matmul groups for block n-1 interleave
into block n's attention stream.
"""

import os
import sys
import time

os.environ.setdefault("NEURON_RT_RESET_CORES", "1")

for _p in ("/opt/trn_rl_repo",):
    if _p not in sys.path:
        sys.path.insert(0, _p)

import numpy as np
import ml_dtypes

import concourse.bass as bass
import concourse.bacc as bacc
import concourse.mybir as mybir
from concourse.tile import TileContext
from concourse import bass_utils

B, L, D = 2, 2048, 2048
H, KVH = 16, 4
HD = D // H            # 128
N_REP = H // KVH       # 4
TP = 4                 # tensor-parallel width (heads)
HQ = H // TP           # 4 query heads per core
SCALE = 1.0 / float(np.sqrt(HD))

F32 = mybir.dt.float32
BF16 = mybir.dt.bfloat16
FP8 = mybir.dt.float8e4
BF = ml_dtypes.bfloat16
F8 = ml_dtypes.float8_e4m3

NPAIR = D // 256       # 8 contraction pairs (K=256 per DoubleRow matmul)
NLT = L // 128         # 16 sequence tiles of 128
NQT = L // 512         # 4 sequence tiles of 512

# fp8 scaling (powers of two; descales folded into existing scale params)
SX = 8.0               # x scale
SW = 512.0             # Wq/Wk/Wv scale
SV = 8.0               # v storage scale
SAO = 16.0             # attn-out storage scale
SWO = 512.0            # Wo scale
ONES_C = SV / SAO      # rowsum ones value: folds Sv->Sao rescale into recip
EXPB = 0.75            # exp bias: at = e^b * w, cancels in normalization
ROPE_DESCALE = 1.0 / (SX * SW)
V_DESCALE = SV / (SX * SW)
OUT_DESCALE = 1.0 / (SAO * SWO)

DR = mybir.MatmulPerfMode.DoubleRow


def build_nc():
    nc = bacc.Bacc(
        "TRN2",
        target_bir_lowering=False,
        debug=False,
        enable_asserts=False,
        num_devices=8,
    )

    xh = nc.dram_tensor("xh", [128, 2 * NPAIR, L], FP8, kind="ExternalInput")
    xl = nc.dram_tensor("xl", [128, 2 * NPAIR, L], FP8, kind="ExternalInput")
    wq = nc.dram_tensor("wq", [128, NPAIR, 2, HQ * HD], FP8, kind="ExternalInput")
    wk = nc.dram_tensor("wk", [128, NPAIR, 2, HD], FP8, kind="ExternalInput")
    wvh = nc.dram_tensor("wvh", [128, NPAIR, 2, HD], FP8, kind="ExternalInput")
    wvl = nc.dram_tensor("wvl", [128, NPAIR, 2, HD], FP8, kind="ExternalInput")
    woh = nc.dram_tensor("woh", [128, 2, 2, D], FP8, kind="ExternalInput")
    wol = nc.dram_tensor("wol", [128, 2, 2, D], FP8, kind="ExternalInput")
    cosT = nc.dram_tensor("cosT", [128, L], BF16, kind="ExternalInput")
    sinT = nc.dram_tensor("sinT", [128, L], BF16, kind="ExternalInput")
    out = nc.dram_tensor("out", [L, D], BF16, kind="ExternalOutput")

    with TileContext(nc) as tc:
        with (
            tc.tile_pool(name="consts", bufs=1) as consts,
            tc.tile_pool(name="xw", bufs=1) as xw,
            tc.tile_pool(name="qkv", bufs=1) as qkv,
            tc.tile_pool(name="rope_t", bufs=4) as rope_t,
            tc.tile_pool(name="at_sb", bufs=3) as at_sb,
            tc.tile_pool(name="norm_sb", bufs=3) as norm_sb,
            tc.tile_pool(name="out_sb", bufs=8) as out_sb,
        ):
            # ---- SBUF-resident inputs ----
            xh_t = xw.tile([128, 2 * NPAIR, L], FP8, tag="xh")
            xl_t = xw.tile([128, 2 * NPAIR, L], FP8, tag="xl")
            wq_t = xw.tile([128, NPAIR, 2, HQ * HD], FP8, tag="wq")
            wk_t = xw.tile([128, NPAIR, 2, HD], FP8, tag="wk")
            wvh_t = xw.tile([128, NPAIR, 2, HD], FP8, tag="wvh")
            wvl_t = xw.tile([128, NPAIR, 2, HD], FP8, tag="wvl")
            woh_t = xw.tile([128, 2, 2, D], FP8, tag="woh")
            wol_t = xw.tile([128, 2, 2, D], FP8, tag="wol")
            cos_t = consts.tile([128, L], BF16, tag="cos")
            sin_t = consts.tile([128, L], BF16, tag="sin")
            # all-ones stationary: the rowsum matmul broadcasts the sum to
            # all 128 output partitions at identical cost (cost = free size),
            # which kills the separate partition-broadcast hop
            ones_c = consts.tile([128, 2, 128], FP8, tag="ones")
            nc.gpsimd.memset(ones_c[:].rearrange("p i l -> p (i l)"), ONES_C)
            expb_t = consts.tile([128, 1], F32, tag="expb")
            nc.gpsimd.memset(expb_t[:], EXPB)

            # loads: wk first (gates K proj), then x chunk pairs hi/lo
            # interleaved (K/Q consume pair c as it lands), weights later.
            nc.gpsimd.dma_start(wk_t[:], wk[:])
            for c in range(NPAIR):
                nc.sync.dma_start(xh_t[:, 2 * c:2 * c + 2, :], xh[:, 2 * c:2 * c + 2, :])
                nc.sync.dma_start(xl_t[:, 2 * c:2 * c + 2, :], xl[:, 2 * c:2 * c + 2, :])
                if c == 0:
                    nc.sync.dma_start(wq_t[:], wq[:])
            nc.sync.dma_start(cos_t[:], cosT[:])
            nc.sync.dma_start(sin_t[:], sinT[:])
            nc.sync.dma_start(wvh_t[:], wvh[:])
            nc.sync.dma_start(wvl_t[:], wvl[:])
            nc.sync.dma_start(woh_t[:], woh[:])
            nc.sync.dma_start(wol_t[:], wol[:])

            # persistent activations
            kT_t = qkv.tile([128, L], BF16, tag="kT")
            qT_t = [qkv.tile([128, L], BF16, tag=f"qT{h}", name=f"qT{h}") for h in range(HQ)]
            vh_t = qkv.tile([128, NPAIR, 2, HD], FP8, tag="vh")
            vl_t = qkv.tile([128, NPAIR, 2, HD], FP8, tag="vl")
            vraw_t = qkv.tile([128, NPAIR, 2, HD], BF16, tag="vraw")
            ao_h = [qkv.tile([128, 2, L], FP8, tag=f"aoh{p}", name=f"aoh{p}") for p in range(2)]
            ao_l = [qkv.tile([128, 2, L], FP8, tag=f"aol{p}", name=f"aol{p}") for p in range(2)]
            # persistent psum-evict staging tiles, one per K/Q proj job, so
            # an evict never waits on the (lazy) rope DVE chain
            qraw_t = [qkv.tile([128, 512], BF16, tag=f"qraw{i}", name=f"qraw{i}")
                      for i in range(20)]
            # manual ring of attention-weight tiles (memset once so the
            # never-exp'd causal prefix regions always read as initialized)
            at_bufs = [qkv.tile([128, 2, 512], FP8, tag=f"at{i}", name=f"at{i}")
                       for i in range(6)]
            for i in range(6):
                nc.gpsimd.memset(at_bufs[i][:].rearrange("p i l -> p (i l)"), 0.0)

            def rope_evict(ps, qraw):
                # one full-width ACT copy frees the psum bank immediately;
                # the rope DVE chain runs later off SBUF (cos/sin are
                # duplicated to 128 partitions so the upper-half multiplies
                # stay base-aligned without a rebase bounce)
                nc.scalar.activation(qraw[:], ps[:],
                                     mybir.ActivationFunctionType.Copy,
                                     scale=ROPE_DESCALE)

            def rope_finish(qraw, dst, sl):
                t0 = rope_t.tile([64, 512], BF16, tag="t0")
                t1 = rope_t.tile([64, 512], BF16, tag="t1")
                t2 = rope_t.tile([64, 512], BF16, tag="t2")
                t3 = rope_t.tile([64, 512], BF16, tag="t3")
                nc.vector.tensor_mul(t0[:], qraw[0:64, :], cos_t[0:64, sl])
                nc.vector.tensor_mul(t1[:], qraw[64:128, :], sin_t[64:128, sl])
                nc.vector.tensor_sub(dst[0:64, sl], t0[:], t1[:])
                nc.vector.tensor_mul(t2[:], qraw[0:64, :], sin_t[0:64, sl])
                nc.vector.tensor_mul(t3[:], qraw[64:128, :], cos_t[64:128, sl])
                nc.vector.tensor_add(dst[64:128, sl], t2[:], t3[:])

            # ---- Projections: batches of 8 concurrent [128,512] psum
            # groups, contraction pair c outer so PE consumes each arriving
            # x chunk immediately. V shares the pool (uses cols 0:HD only).
            jobs = []
            for nk in range(NQT):
                jobs.append(("k", 0, nk))
            for nq in range(NQT):
                jobs.append(("q", 0, nq))
            for h in range(1, 3):
                for nq in range(NQT):
                    jobs.append(("q", h, nq))
            for lt in range(NLT):
                jobs.append(("v", 0, lt))
            for nq in range(NQT):
                jobs.append(("q", 3, nq))

            qraw_i = 0
            deferred_ropes = []

            def proj_mms(ps, job, c, term):
                kind, h, idx = job
                st = c == 0 and term == "hi"
                if kind == "v":
                    # 3-term: (xh,wh)+(xl,wh) then (xh,wl)
                    lsl = slice(idx * 128, (idx + 1) * 128)
                    if term == "hi":
                        nc.tensor.matmul(
                            ps[:, 0:HD],
                            xh_t[:, 2 * c:2 * c + 2, lsl],
                            wvh_t[:, c, :, :],
                            start=st, stop=False,
                            perf_mode=DR, skip_group_check=True,
                        )
                        nc.tensor.matmul(
                            ps[:, 0:HD],
                            xl_t[:, 2 * c:2 * c + 2, lsl],
                            wvh_t[:, c, :, :],
                            start=False, stop=False,
                            perf_mode=DR, skip_group_check=True,
                        )
                    else:
                        nc.tensor.matmul(
                            ps[:, 0:HD],
                            xh_t[:, 2 * c:2 * c + 2, lsl],
                            wvl_t[:, c, :, :],
                            start=False,
                            stop=(c == NPAIR - 1),
                            perf_mode=DR, skip_group_check=True,
                        )
                    return
                xt = xh_t if term == "hi" else xl_t
                sp = c == NPAIR - 1 and term == "lo"
                sl = slice(idx * 512, (idx + 1) * 512)
                if kind == "k":
                    lhsT = wk_t[:, c, :, :]
                else:
                    lhsT = wq_t[:, c, :, h * HD:(h + 1) * HD]
                nc.tensor.matmul(
                    ps[:], lhsT, xt[:, 2 * c:2 * c + 2, sl],
                    start=st, stop=sp, perf_mode=DR,
                    skip_group_check=True,
                )

            def proj_finish(ps, job):
                nonlocal qraw_i
                kind, h, idx = job
                if kind == "v":
                    m, i = divmod(idx, 2)
                    nc.scalar.activation(
                        vh_t[:, m, i, :], ps[:, 0:HD],
                        mybir.ActivationFunctionType.Copy,
                        scale=V_DESCALE)
                    nc.scalar.activation(
                        vraw_t[:, m, i, :], ps[:, 0:HD],
                        mybir.ActivationFunctionType.Copy,
                        scale=V_DESCALE)
                    # residual on Pool: keeps DVE (rope backlog) and
                    # psum (freed by the ACT copies) off the V path
                    nc.gpsimd.tensor_sub(
                        vl_t[:, m, i, :], vraw_t[:, m, i, :],
                        vh_t[:, m, i, :])
                else:
                    dst = kT_t if kind == "k" else qT_t[h]
                    sl = slice(idx * 512, (idx + 1) * 512)
                    rope_evict(ps, qraw_t[qraw_i])
                    if kind == "k" or h == 0:
                        rope_finish(qraw_t[qraw_i], dst, sl)
                    else:
                        deferred_ropes.append((qraw_i, dst, sl, h, idx))
                    qraw_i += 1

            def run_projections():
                # batch A (first 8 jobs): chunk-major, paced by the x stream.
                # later batches: job-major (x resident) so evicts/copies
                # overlap the next job's matmuls.
                with tc.tile_pool(name="proj_ps", bufs=8, space="PSUM") as pp:
                    batch = jobs[0:8]
                    tiles = [pp.tile([128, 512], F32, tag="proj", name=f"pj0_{i}")
                             for i in range(len(batch))]
                    for c in range(NPAIR):
                        for term in ("hi", "lo"):
                            for ps, job in zip(tiles, batch):
                                proj_mms(ps, job, c, term)
                    for ps, job in zip(tiles, batch):
                        proj_finish(ps, job)
                    for jn, job in enumerate(jobs[8:]):
                        ps = pp.tile([128, 512], F32, tag="proj", name=f"pjl{jn}")
                        for c in range(NPAIR):
                            proj_mms(ps, job, c, "hi")
                            proj_mms(ps, job, c, "lo")
                        proj_finish(ps, job)

            run_projections()

            # ---- attention + output projection ----
            # psum: s_ps 2x2 banks, o_ps 1, sum_ps 1, wo_ps 2 = 8 banks
            with (
                tc.tile_pool(name="s_ps", bufs=2, space="PSUM") as s_ps,
                tc.tile_pool(name="sum_ps", bufs=1, space="PSUM") as sum_ps,
                tc.tile_pool(name="o_ps", bufs=1, space="PSUM") as o_ps,
                tc.tile_pool(name="wo_ps", bufs=2, space="PSUM") as wo_ps,
            ):
                ncopy = 0

                def wo_job(lt, no, tail=False):
                    nonlocal ncopy
                    lsl = slice(lt * 128, (lt + 1) * 128)
                    osl = slice(no * 512, (no + 1) * 512)
                    ps = wo_ps.tile([128, 512], F32, tag="wo")
                    k = 0
                    for hp in range(2):
                        for at_, wt in ((ao_h[hp], woh_t), (ao_l[hp], woh_t),
                                        (ao_h[hp], wol_t)):
                            nc.tensor.matmul(
                                ps[:], at_[:, :, lsl], wt[:, hp, :, osl],
                                start=(k == 0), stop=(k == 5),
                                perf_mode=DR, skip_group_check=True,
                            )
                            k += 1
                    ot = out_sb.tile([128, 512], BF16, tag="out")
                    if tail and ncopy % 2 == 1:
                        nc.scalar.activation(ot[:], ps[:],
                                             mybir.ActivationFunctionType.Copy,
                                             scale=OUT_DESCALE)
                    else:
                        nc.vector.tensor_scalar_mul(ot[:], ps[:], OUT_DESCALE)
                    ncopy += 1
                    nc.sync.dma_start(out[lsl, osl], ot[:])

                # qsl-major so the chain a head needs next is emitted first
                deferred_ropes.sort(key=lambda r: (r[4], r[3]))

                def pop_rope(n=1):
                    for _ in range(n):
                        if deferred_ropes:
                            qi, dst, sl, _, _ = deferred_ropes.pop(0)
                            rope_finish(qraw_t[qi], dst, sl)

                wo_queue = []
                at_ctr = 0
                pop_rope(1)
                for nq in range(NQT):
                    qb = nq * 512
                    qsl = slice(qb, qb + 512)
                    ngrp = 2 * (nq + 1)
                    # spread previous block's wo jobs through this block's
                    # attention groups (hides exp latency + out-copy drain)
                    total_grps = HQ * ngrp
                    nwo = len(wo_queue)
                    gctr = 0
                    wo_done = 0
                    for h in range(HQ):
                        hp, hi = divmod(h, 2)
                        psq = sum_ps.tile([128, 512], F32, tag="rowsum")
                        pso = o_ps.tile([128, 512], F32, tag="aout")
                        pending = []  # (at tile, group idx) awaiting av mms

                        for g in range(ngrp):
                            ps = s_ps.tile([128, 2, 512], F32, tag="sc")
                            at = at_bufs[at_ctr % 6]
                            at_ctr += 1
                            for i in range(2):
                                mk = 2 * g + i
                                j = mk - 4 * nq
                                ksl = slice(mk * 128, (mk + 1) * 128)
                                if j > 0:
                                    # diagonal: suffix-restricted matmul
                                    nc.tensor.matmul(
                                        ps[:, i, 128 * j:512],
                                        kT_t[:, ksl],
                                        qT_t[h][:, qb + 128 * j:qb + 512],
                                        start=True, stop=True,
                                    )
                                else:
                                    nc.tensor.matmul(
                                        ps[:, i, :], kT_t[:, ksl], qT_t[h][:, qsl],
                                        start=True, stop=True,
                                    )
                            # emit previous group's av/rowsum now: PE does
                            # this group's scores while ACT exps prev group
                            if len(pending) >= 1:
                                pat, pg = pending.pop(0)
                                first = pg == 0
                                m = pg - 2 * nq  # diag pair index if >= 0
                                ssl = slice(256 * m, 512) if m > 0 else slice(0, 512)
                                nc.tensor.matmul(
                                    psq[:, ssl], ones_c[:], pat[:, :, ssl],
                                    start=first, stop=(pg == ngrp - 1),
                                    perf_mode=DR, skip_group_check=True,
                                )
                                nc.tensor.matmul(
                                    pso[:, ssl], vh_t[:, pg, :, :], pat[:, :, ssl],
                                    start=first, stop=False,
                                    perf_mode=DR, skip_group_check=True,
                                )
                                nc.tensor.matmul(
                                    pso[:, ssl], vl_t[:, pg, :, :], pat[:, :, ssl],
                                    start=False, stop=(pg == ngrp - 1),
                                    perf_mode=DR, skip_group_check=True,
                                )
                            if 2 * g >= 4 * nq:
                                # diagonal group: exp only valid suffixes,
                                # affine zeroes stale prefix + causal triangle
                                m = g - 2 * nq
                                for i in range(2):
                                    j = 2 * g + i - 4 * nq
                                    nc.scalar.activation(
                                        at[:, i, 128 * j:512],
                                        ps[:, i, 128 * j:512],
                                        mybir.ActivationFunctionType.Exp,
                                        scale=SCALE, bias=expb_t[:],
                                    )
                                    # av/rowsum read only cols >= 256m; zero
                                    # the boundary triangle (and, for i=1,
                                    # the stale 128-col prefix) of that region
                                    w0 = 256 * m
                                    nc.gpsimd.affine_select(
                                        out=at[:, i, w0:128 * (j + 1)],
                                        in_=at[:, i, w0:128 * (j + 1)],
                                        compare_op=mybir.AluOpType.is_ge,
                                        fill=0.0, base=w0 - 128 * j,
                                        pattern=[[1, 128 * (j + 1) - w0]],
                                        channel_multiplier=-1,
                                    )
                            else:
                                nc.scalar.activation(
                                    at[:].rearrange("p i l -> p (i l)"),
                                    ps[:].rearrange("p i l -> p (i l)"),
                                    mybir.ActivationFunctionType.Exp,
                                    scale=SCALE, bias=expb_t[:],
                                )
                            pending.append((at, g))
                            gctr += 1
                            while wo_queue and wo_done < gctr * nwo // total_grps:
                                wo_job(*wo_queue.pop(0))
                                wo_done += 1
                        # flush remaining groups' av
                        while pending:
                            pat, pg = pending.pop(0)
                            m = pg - 2 * nq
                            ssl = slice(256 * m, 512) if m > 0 else slice(0, 512)
                            last = not pending
                            nc.tensor.matmul(
                                psq[:, ssl], ones_c[:], pat[:, :, ssl],
                                start=(pg == 0), stop=last,
                                perf_mode=DR, skip_group_check=True,
                            )
                            nc.tensor.matmul(
                                pso[:, ssl], vh_t[:, pg, :, :], pat[:, :, ssl],
                                start=(pg == 0), stop=False,
                                perf_mode=DR, skip_group_check=True,
                            )
                            nc.tensor.matmul(
                                pso[:, ssl], vl_t[:, pg, :, :], pat[:, :, ssl],
                                start=False, stop=last,
                                perf_mode=DR, skip_group_check=True,
                            )
                        # normalization: recip (DVE), partition broadcast
                        # (gpsimd), scale+store hi/lo attn-out
                        rbs = norm_sb.tile([128, 512], F32, tag="rbs")
                        nc.vector.reciprocal(rbs[:], psq[:])
                        aot = norm_sb.tile([128, 512], BF16, tag="aot")
                        nc.vector.tensor_mul(aot[:], pso[:], rbs[:])
                        nc.gpsimd.tensor_copy(ao_h[hp][:, hi, qsl], aot[:])
                        nc.gpsimd.tensor_sub(ao_l[hp][:, hi, qsl], aot[:],
                                             ao_h[hp][:, hi, qsl])
                        pop_rope(1)
                    while wo_queue:
                        wo_job(*wo_queue.pop(0))
                    for lt in range(4 * nq, 4 * nq + 4):
                        for no in range(NQT):
                            wo_queue.append((lt, no))
                while wo_queue:
                    wo_job(*wo_queue.pop(0), tail=True)

    nc.compile()
    return nc


_ROPE_PERM = np.concatenate([np.arange(0, HD, 2), np.arange(1, HD, 2)])


def _q8(a):
    return np.clip(a, -240.0, 240.0).astype(F8)


def _prep_inputs(x, freqs_cos, freqs_sin, Wq, Wk, Wv, Wo):
    """Build the 8 per-core input maps (numpy, host-side)."""
    x = np.asarray(x, np.float32)
    cosT = np.ascontiguousarray(
        np.tile(np.asarray(freqs_cos, np.float32).T, (2, 1))).astype(BF)
    sinT = np.ascontiguousarray(
        np.tile(np.asarray(freqs_sin, np.float32).T, (2, 1))).astype(BF)
    Wq = np.asarray(Wq, np.float32)
    Wk = np.asarray(Wk, np.float32)
    Wv = np.asarray(Wv, np.float32)
    Wo = np.asarray(Wo, np.float32)

    # x layout: [p, 2c+i, l] = x[b, l, 256c+128i+p], scaled/hi-lo fp8
    def xlayout(xb):
        # xb [L, D] -> [128, 16, L]
        t = (SX * xb.T).reshape(2 * NPAIR, 128, L).transpose(1, 0, 2)
        hi = _q8(t)
        lo = _q8(t - hi.astype(np.float32))
        return np.ascontiguousarray(hi), np.ascontiguousarray(lo)

    xh_b = [xlayout(x[b]) for b in range(B)]

    def wlayout(W, ncol):
        # W [D, ncol] -> [128, NPAIR, 2, ncol] (rows 256c+128i+p)
        return np.ascontiguousarray(
            W.reshape(NPAIR, 2, 128, ncol).transpose(2, 0, 1, 3))

    in_maps = []
    for core in range(8):
        b, t = divmod(core, TP)
        wq_c = Wq[:, t * HQ * HD:(t + 1) * HQ * HD].reshape(D, HQ, HD)
        wq_c = wq_c[:, :, _ROPE_PERM].reshape(D, HQ * HD)
        wk_c = Wk[:, t * HD:(t + 1) * HD][:, _ROPE_PERM]
        wv_c = Wv[:, t * HD:(t + 1) * HD]
        wo_c = Wo[t * HQ * HD:(t + 1) * HQ * HD, :]  # [512, D]

        wvs = SW * wlayout(wv_c, HD)
        wvh_c = _q8(wvs)
        wvl_c = _q8(wvs - wvh_c.astype(np.float32))
        # wo layout: [p, hp, i, d] = Wo[128*(2hp+i)+p, d]
        wos = SWO * np.ascontiguousarray(
            wo_c.reshape(2, 2, 128, D).transpose(2, 0, 1, 3))
        woh_c = _q8(wos)
        wol_c = _q8(wos - woh_c.astype(np.float32))

        in_maps.append({
            "xh": xh_b[b][0],
            "xl": xh_b[b][1],
            "wq": _q8(SW * wlayout(wq_c, HQ * HD)),
            "wk": _q8(SW * wlayout(wk_c, HD)),
            "wvh": wvh_c,
            "wvl": wvl_c,
            "woh": woh_c,
            "wol": wol_c,
            "cosT": cosT,
            "sinT": sinT,
        })
    return in_maps


_NC_CACHE = None


def run(inputs, trace=False, trace_kwargs=None):
    global _NC_CACHE
    if _NC_CACHE is None:
        _NC_CACHE = build_nc()
    nc = _NC_CACHE
    in_maps = _prep_inputs(
        inputs["x"], inputs["freqs_cos"], inputs["freqs_sin"],
        inputs["Wq"], inputs["Wk"], inputs["Wv"], inputs["Wo"],
    )
    res = None
    last_err = None
    for attempt in range(4):
        if attempt:
            time.sleep(5.0 * attempt)  # transient device wedges clear with a pause
        try:
            res = bass_utils.run_bass_kernel_spmd(
                nc, in_maps, core_ids=list(range(8)),
                trace=trace and attempt == 0, **(trace_kwargs or {}),
            )
            break
        except ModuleNotFoundError:
            try:
                res = bass_utils.run_bass_kernel_spmd(
                    nc, in_maps, core_ids=list(range(8)), trace=False,
                )
                break
            except Exception as e:  # transient device errors: retry
                last_err = e
        except Exception as e:
            last_err = e
    if res is None:
        raise last_err
    partials = [r["out"] for r in res.results]
    out = np.empty((B, L, D), np.float32)
    for b in range(B):
        acc = partials[b * TP].astype(np.float32)
        for t in range(1, TP):
            acc = acc + partials[b * TP + t]
        out[b] = acc
    # exact host-side bias folds: +bo, and +bv @ Wo (softmax rows sum to 1).
    bo = np.asarray(inputs["bo"], np.float32)
    bv = np.asarray(inputs["bv"], np.float32)
    Wo = np.asarray(inputs["Wo"], np.float32)
    bias = bo + np.repeat(bv.reshape(KVH, HD), N_REP, axis=0).reshape(-1) @ Wo
    out += bias[None, None, :]
    return out, res


def kernel(**inputs) -> np.ndarray:
    out, _ = run(inputs, trace=False)
    return out


if __name__ == "__main__":
    pass


# revision 55
# speedup vs baseline: 1.0168x; 1.0168x over previous
"""Trainium2 Bass kernel for GQA causal attention (B=2, L=2048, D=2048, H=16, KVH=4).

Sharding: 8 cores = 2-way data-parallel (batch) x 4-way tensor-parallel (heads).
Each core handles one batch element, 4 query heads, and their shared KV head.
Wo is row-sharded; the host sums the 4 partial outputs per batch.

Mixed-precision fp8 (e4m3) with DoubleRow matmuls, tuned against the CoreSim
cost model (DoubleRow = 0.5 cycles/row with K=256 per instruction = 4x bf16):
  - Q/K projections: 2-term hi-lo split of x (xh*w + xl*w), w plain fp8.
    Residual error = w quantization only, softmax-dampened downstream.
  - V projection: 3-term hi-lo (xh*wh + xl*wh + xh*wl) -> near-exact.
  - Scores: bf16 (full accuracy); causal diagonal via suffix-restricted
    matmuls + suffix exps + affine_select on the boundary windows (the
    attn@v/rowsum matmuls also suffix-skip the all-zero masked columns).
  - Attention weights (exp output): plain fp8 with +0.75 exp bias (folded
    into softmax, cancels exactly in the normalization).
  - rowsum + attn@v: DoubleRow over k-tile pairs; v is hi-lo (2 accumulating
    matmuls); the "ones" vector carries Sv/Sao so normalization scales land
    for free.
  - Wo: 3-term hi-lo fp8.
All fp8 tensors are pre-scaled into e4m3's normal range (x*8, W*512, v*8,
ao*16); descales fold into ACT copy scales, the exp scale, and the ones value.

Softmax normalization: the rowsum ones-matmul uses an all-ones [128,2,128]
stationary so every output partition carries the sum (matmul cost depends
only on free size), then reciprocal on DVE and one multiply - no separate
broadcast step.

Scheduling: projections stream x in hi/lo chunk pairs with 8 concurrent
psum groups consuming each chunk as it lands; psums are evicted by a single
ACT copy into persistent staging tiles (cos/sin duplicated to 128 partitions
so rope needs no rebase bounce) and the rope DVE chains for heads 1-3 are
deferred into the attention phase; Wo matmul groups for block n-1 interleave
into block n's attention stream.
"""

import os
import sys
import time

os.environ.setdefault("NEURON_RT_RESET_CORES", "1")

for _p in ("/opt/trn_rl_repo",):
    if _p not in sys.path:
        sys.path.insert(0, _p)

import numpy as np
import ml_dtypes

import concourse.bass as bass
import concourse.bacc as bacc
import concourse.mybir as mybir
from concourse.tile import TileContext
from concourse import bass_utils

B, L, D = 2, 2048, 2048
H, KVH = 16, 4
HD = D // H            # 128
N_REP = H // KVH       # 4
TP = 4                 # tensor-parallel width (heads)
HQ = H // TP           # 4 query heads per core
SCALE = 1.0 / float(np.sqrt(HD))

F32 = mybir.dt.float32
BF16 = mybir.dt.bfloat16
FP8 = mybir.dt.float8e4
BF = ml_dtypes.bfloat16
F8 = ml_dtypes.float8_e4m3

NPAIR = D // 256       # 8 contraction pairs (K=256 per DoubleRow matmul)
NLT = L // 128         # 16 sequence tiles of 128
NQT = L // 512         # 4 sequence tiles of 512

# fp8 scaling (powers of two; descales folded into existing scale params)
SX = 8.0               # x scale
SW = 512.0             # Wq/Wk/Wv scale
SV = 8.0               # v storage scale
SAO = 16.0             # attn-out storage scale
SWO = 512.0            # Wo scale
ONES_C = SV / SAO      # rowsum ones value: folds Sv->Sao rescale into recip
EXPB = 0.75            # exp bias: at = e^b * w, cancels in normalization
ROPE_DESCALE = 1.0 / (SX * SW)
V_DESCALE = SV / (SX * SW)
OUT_DESCALE = 1.0 / (SAO * SWO)

DR = mybir.MatmulPerfMode.DoubleRow


def build_nc():
    nc = bacc.Bacc(
        "TRN2",
        target_bir_lowering=False,
        debug=False,
        enable_asserts=False,
        num_devices=8,
    )

    xh = nc.dram_tensor("xh", [128, 2 * NPAIR, L], FP8, kind="ExternalInput")
    xl = nc.dram_tensor("xl", [128, 2 * NPAIR, L], FP8, kind="ExternalInput")
    wq = nc.dram_tensor("wq", [128, NPAIR, 2, HQ * HD], FP8, kind="ExternalInput")
    wk = nc.dram_tensor("wk", [128, NPAIR, 2, HD], FP8, kind="ExternalInput")
    wvh = nc.dram_tensor("wvh", [128, NPAIR, 2, HD], FP8, kind="ExternalInput")
    wvl = nc.dram_tensor("wvl", [128, NPAIR, 2, HD], FP8, kind="ExternalInput")
    woh = nc.dram_tensor("woh", [128, 2, 2, D], FP8, kind="ExternalInput")
    wol = nc.dram_tensor("wol", [128, 2, 2, D], FP8, kind="ExternalInput")
    cosT = nc.dram_tensor("cosT", [128, L], BF16, kind="ExternalInput")
    sinT = nc.dram_tensor("sinT", [128, L], BF16, kind="ExternalInput")
    out = nc.dram_tensor("out", [L, D], BF16, kind="ExternalOutput")

    with TileContext(nc) as tc:
        with (
            tc.tile_pool(name="consts", bufs=1) as consts,
            tc.tile_pool(name="xw", bufs=1) as xw,
            tc.tile_pool(name="qkv", bufs=1) as qkv,
            tc.tile_pool(name="rope_t", bufs=4) as rope_t,
            tc.tile_pool(name="at_sb", bufs=3) as at_sb,
            tc.tile_pool(name="norm_sb", bufs=3) as norm_sb,
            tc.tile_pool(name="out_sb", bufs=6) as out_sb,
        ):
            # ---- SBUF-resident inputs ----
            xh_t = xw.tile([128, 2 * NPAIR, L], FP8, tag="xh")
            xl_t = xw.tile([128, 2 * NPAIR, L], FP8, tag="xl")
            wq_t = xw.tile([128, NPAIR, 2, HQ * HD], FP8, tag="wq")
            wk_t = xw.tile([128, NPAIR, 2, HD], FP8, tag="wk")
            wvh_t = xw.tile([128, NPAIR, 2, HD], FP8, tag="wvh")
            wvl_t = xw.tile([128, NPAIR, 2, HD], FP8, tag="wvl")
            woh_t = xw.tile([128, 2, 2, D], FP8, tag="woh")
            wol_t = xw.tile([128, 2, 2, D], FP8, tag="wol")
            cos_t = consts.tile([128, L], BF16, tag="cos")
            sin_t = consts.tile([128, L], BF16, tag="sin")
            # all-ones stationary: the rowsum matmul broadcasts the sum to
            # all 128 output partitions at identical cost (cost = free size),
            # which kills the separate partition-broadcast hop
            ones_c = consts.tile([128, 2, 128], FP8, tag="ones")
            nc.gpsimd.memset(ones_c[:].rearrange("p i l -> p (i l)"), ONES_C)
            expb_t = consts.tile([128, 1], F32, tag="expb")
            nc.gpsimd.memset(expb_t[:], EXPB)

            # loads: wk first (gates K proj), then x chunk pairs hi/lo
            # interleaved (K/Q consume pair c as it lands), weights later.
            nc.gpsimd.dma_start(wk_t[:], wk[:])
            for c in range(NPAIR):
                nc.sync.dma_start(xh_t[:, 2 * c:2 * c + 2, :], xh[:, 2 * c:2 * c + 2, :])
                nc.sync.dma_start(xl_t[:, 2 * c:2 * c + 2, :], xl[:, 2 * c:2 * c + 2, :])
                if c == 0:
                    nc.sync.dma_start(wq_t[:], wq[:])
            nc.sync.dma_start(cos_t[:], cosT[:])
            nc.sync.dma_start(sin_t[:], sinT[:])
            nc.sync.dma_start(wvh_t[:], wvh[:])
            nc.sync.dma_start(wvl_t[:], wvl[:])
            nc.sync.dma_start(woh_t[:], woh[:])
            nc.sync.dma_start(wol_t[:], wol[:])

            # persistent activations
            kT_t = qkv.tile([128, L], BF16, tag="kT")
            kT8_t = qkv.tile([64, 2, L], FP8, tag="kT8")
            qT_t = [None, None] + [qkv.tile([128, L], BF16, tag=f"qT{h}", name=f"qT{h}")
                                   for h in range(2, HQ)]
            qT8_t = [qkv.tile([64, 2, L], FP8, tag=f"qT8{h}", name=f"qT8{h}")
                     for h in range(2)]
            vh_t = qkv.tile([128, NPAIR, 2, HD], FP8, tag="vh")
            vl_t = qkv.tile([128, NPAIR, 2, HD], FP8, tag="vl")
            vraw_t = qkv.tile([128, NPAIR, 2, HD], BF16, tag="vraw")
            ao_h = [qkv.tile([128, 2, L], FP8, tag=f"aoh{p}", name=f"aoh{p}") for p in range(2)]
            ao_l = [qkv.tile([128, 2, L], FP8, tag=f"aol{p}", name=f"aol{p}") for p in range(2)]
            # persistent psum-evict staging tiles, one per K/Q proj job, so
            # an evict never waits on the (lazy) rope DVE chain
            qraw_t = [qkv.tile([128, 512], BF16, tag=f"qraw{i}", name=f"qraw{i}")
                      for i in range(20)]
            # manual ring of attention-weight tiles (memset once so the
            # never-exp'd causal prefix regions always read as initialized)
            at_bufs = [qkv.tile([128, 2, 512], FP8, tag=f"at{i}", name=f"at{i}")
                       for i in range(6)]
            for i in range(6):
                nc.gpsimd.memset(at_bufs[i][:].rearrange("p i l -> p (i l)"), 0.0)

            def rope_evict(ps, qraw):
                # one full-width ACT copy frees the psum bank immediately;
                # the rope DVE chain runs later off SBUF (cos/sin are
                # duplicated to 128 partitions so the upper-half multiplies
                # stay base-aligned without a rebase bounce)
                nc.scalar.activation(qraw[:], ps[:],
                                     mybir.ActivationFunctionType.Copy,
                                     scale=ROPE_DESCALE)

            def rope_finish(qraw, dst, sl, dst8=None):
                t0 = rope_t.tile([64, 512], BF16, tag="t0")
                t1 = rope_t.tile([64, 512], BF16, tag="t1")
                t2 = rope_t.tile([64, 512], BF16, tag="t2")
                t3 = rope_t.tile([64, 512], BF16, tag="t3")
                nc.vector.tensor_mul(t0[:], qraw[0:64, :], cos_t[0:64, sl])
                nc.vector.tensor_mul(t1[:], qraw[64:128, :], sin_t[64:128, sl])
                nc.vector.tensor_mul(t2[:], qraw[0:64, :], sin_t[0:64, sl])
                nc.vector.tensor_mul(t3[:], qraw[64:128, :], cos_t[64:128, sl])
                if dst is None:
                    # fp8-score head: only the DoubleRow hd-split layout
                    nc.vector.tensor_sub(dst8[:, 0, sl], t0[:], t1[:])
                    nc.vector.tensor_add(dst8[:, 1, sl], t2[:], t3[:])
                else:
                    nc.vector.tensor_sub(dst[0:64, sl], t0[:], t1[:])
                    nc.vector.tensor_add(dst[64:128, sl], t2[:], t3[:])
                    if dst8 is not None:
                        # K: also stage an fp8 copy for the fp8-score heads
                        nc.gpsimd.tensor_copy(dst8[:, 0, sl], dst[0:64, sl])
                        nc.gpsimd.tensor_copy(dst8[:, 1, sl], dst[64:128, sl])

            # ---- Projections: batches of 8 concurrent [128,512] psum
            # groups, contraction pair c outer so PE consumes each arriving
            # x chunk immediately. V shares the pool (uses cols 0:HD only).
            jobs = []
            for nk in range(NQT):
                jobs.append(("k", 0, nk))
            for nq in range(NQT):
                jobs.append(("q", 0, nq))
            for h in range(1, 3):
                for nq in range(NQT):
                    jobs.append(("q", h, nq))
            for lt in range(NLT):
                jobs.append(("v", 0, lt))
            for nq in range(NQT):
                jobs.append(("q", 3, nq))

            qraw_i = 0
            deferred_ropes = []

            def proj_mms(ps, job, c, term):
                kind, h, idx = job
                st = c == 0 and term == "hi"
                if kind == "v":
                    # 3-term: (xh,wh)+(xl,wh) then (xh,wl)
                    lsl = slice(idx * 128, (idx + 1) * 128)
                    if term == "hi":
                        nc.tensor.matmul(
                            ps[:, 0:HD],
                            xh_t[:, 2 * c:2 * c + 2, lsl],
                            wvh_t[:, c, :, :],
                            start=st, stop=False,
                            perf_mode=DR, skip_group_check=True,
                        )
                        nc.tensor.matmul(
                            ps[:, 0:HD],
                            xl_t[:, 2 * c:2 * c + 2, lsl],
                            wvh_t[:, c, :, :],
                            start=False, stop=False,
                            perf_mode=DR, skip_group_check=True,
                        )
                    else:
                        nc.tensor.matmul(
                            ps[:, 0:HD],
                            xh_t[:, 2 * c:2 * c + 2, lsl],
                            wvl_t[:, c, :, :],
                            start=False,
                            stop=(c == NPAIR - 1),
                            perf_mode=DR, skip_group_check=True,
                        )
                    return
                xt = xh_t if term == "hi" else xl_t
                sp = c == NPAIR - 1 and term == "lo"
                sl = slice(idx * 512, (idx + 1) * 512)
                if kind == "k":
                    lhsT = wk_t[:, c, :, :]
                else:
                    lhsT = wq_t[:, c, :, h * HD:(h + 1) * HD]
                nc.tensor.matmul(
                    ps[:], lhsT, xt[:, 2 * c:2 * c + 2, sl],
                    start=st, stop=sp, perf_mode=DR,
                    skip_group_check=True,
                )

            def proj_finish(ps, job):
                nonlocal qraw_i
                kind, h, idx = job
                if kind == "v":
                    m, i = divmod(idx, 2)
                    nc.scalar.activation(
                        vh_t[:, m, i, :], ps[:, 0:HD],
                        mybir.ActivationFunctionType.Copy,
                        scale=V_DESCALE)
                    nc.scalar.activation(
                        vraw_t[:, m, i, :], ps[:, 0:HD],
                        mybir.ActivationFunctionType.Copy,
                        scale=V_DESCALE)
                    # residual on Pool: keeps DVE (rope backlog) and
                    # psum (freed by the ACT copies) off the V path
                    nc.gpsimd.tensor_sub(
                        vl_t[:, m, i, :], vraw_t[:, m, i, :],
                        vh_t[:, m, i, :])
                else:
                    sl = slice(idx * 512, (idx + 1) * 512)
                    rope_evict(ps, qraw_t[qraw_i])
                    if kind == "k":
                        rope_finish(qraw_t[qraw_i], kT_t, sl, dst8=kT8_t)
                    elif h == 0:
                        rope_finish(qraw_t[qraw_i], None, sl, dst8=qT8_t[0])
                    else:
                        deferred_ropes.append((qraw_i, h, sl, h, idx))
                    qraw_i += 1

            def run_projections():
                # batch A (first 8 jobs): chunk-major, paced by the x stream.
                # later batches: job-major (x resident) so evicts/copies
                # overlap the next job's matmuls.
                with tc.tile_pool(name="proj_ps", bufs=8, space="PSUM") as pp:
                    batch = jobs[0:8]
                    tiles = [pp.tile([128, 512], F32, tag="proj", name=f"pj0_{i}")
                             for i in range(len(batch))]
                    for c in range(NPAIR):
                        for term in ("hi", "lo"):
                            for ps, job in zip(tiles, batch):
                                proj_mms(ps, job, c, term)
                    for ps, job in zip(tiles, batch):
                        proj_finish(ps, job)
                    for jn, job in enumerate(jobs[8:]):
                        ps = pp.tile([128, 512], F32, tag="proj", name=f"pjl{jn}")
                        for c in range(NPAIR):
                            proj_mms(ps, job, c, "hi")
                            proj_mms(ps, job, c, "lo")
                        proj_finish(ps, job)

            run_projections()

            # ---- attention + output projection ----
            # psum: s_ps 2x2 banks, o_ps 1, sum_ps 1, wo_ps 2 = 8 banks
            with (
                tc.tile_pool(name="s_ps", bufs=2, space="PSUM") as s_ps,
                tc.tile_pool(name="sum_ps", bufs=1, space="PSUM") as sum_ps,
                tc.tile_pool(name="o_ps", bufs=1, space="PSUM") as o_ps,
                tc.tile_pool(name="wo_ps", bufs=2, space="PSUM") as wo_ps,
            ):
                ncopy = 0

                def wo_job(lt, no, tail=False):
                    nonlocal ncopy
                    lsl = slice(lt * 128, (lt + 1) * 128)
                    osl = slice(no * 512, (no + 1) * 512)
                    ps = wo_ps.tile([128, 512], F32, tag="wo")
                    k = 0
                    for hp in range(2):
                        for at_, wt in ((ao_h[hp], woh_t), (ao_l[hp], woh_t),
                                        (ao_h[hp], wol_t)):
                            nc.tensor.matmul(
                                ps[:], at_[:, :, lsl], wt[:, hp, :, osl],
                                start=(k == 0), stop=(k == 5),
                                perf_mode=DR, skip_group_check=True,
                            )
                            k += 1
                    ot = out_sb.tile([128, 512], BF16, tag="out")
                    if tail and ncopy % 2 == 1:
                        nc.scalar.activation(ot[:], ps[:],
                                             mybir.ActivationFunctionType.Copy,
                                             scale=OUT_DESCALE)
                    else:
                        nc.vector.tensor_scalar_mul(ot[:], ps[:], OUT_DESCALE)
                    ncopy += 1
                    nc.sync.dma_start(out[lsl, osl], ot[:])

                # qsl-major so the chain a head needs next is emitted first
                deferred_ropes.sort(key=lambda r: (r[4], r[3]))

                def pop_rope(n=1):
                    for _ in range(n):
                        if deferred_ropes:
                            qi, hh, sl, _, _ = deferred_ropes.pop(0)
                            if hh < 2:
                                rope_finish(qraw_t[qi], None, sl, dst8=qT8_t[hh])
                            else:
                                rope_finish(qraw_t[qi], qT_t[hh], sl)

                wo_queue = []
                at_ctr = 0
                pop_rope(1)
                for nq in range(NQT):
                    qb = nq * 512
                    qsl = slice(qb, qb + 512)
                    ngrp = 2 * (nq + 1)
                    # spread previous block's wo jobs through this block's
                    # attention groups (hides exp latency + out-copy drain)
                    total_grps = HQ * ngrp
                    nwo = len(wo_queue)
                    gctr = 0
                    wo_done = 0
                    for h in range(HQ):
                        hp, hi = divmod(h, 2)
                        psq = sum_ps.tile([128, 512], F32, tag="rowsum")
                        pso = o_ps.tile([128, 512], F32, tag="aout")
                        pending = []  # (at tile, group idx) awaiting av mms

                        for g in range(ngrp):
                            ps = s_ps.tile([128, 2, 512], F32, tag="sc")
                            at = at_bufs[at_ctr % 6]
                            at_ctr += 1
                            for i in range(2):
                                mk = 2 * g + i
                                j = mk - 4 * nq
                                ksl = slice(mk * 128, (mk + 1) * 128)
                                w0 = 128 * j if j > 0 else 0
                                if h < 2:
                                    nc.tensor.matmul(
                                        ps[:, i, w0:512],
                                        kT8_t[:, :, ksl],
                                        qT8_t[h][:, :, qb + w0:qb + 512],
                                        start=True, stop=True,
                                        perf_mode=DR, skip_group_check=True,
                                    )
                                elif j > 0:
                                    # diagonal: suffix-restricted matmul
                                    nc.tensor.matmul(
                                        ps[:, i, 128 * j:512],
                                        kT_t[:, ksl],
                                        qT_t[h][:, qb + 128 * j:qb + 512],
                                        start=True, stop=True,
                                    )
                                else:
                                    nc.tensor.matmul(
                                        ps[:, i, :], kT_t[:, ksl], qT_t[h][:, qsl],
                                        start=True, stop=True,
                                    )
                            # emit previous group's av/rowsum now: PE does
                            # this group's scores while ACT exps prev group
                            if len(pending) >= 1:
                                pat, pg = pending.pop(0)
                                first = pg == 0
                                m = pg - 2 * nq  # diag pair index if >= 0
                                ssl = slice(256 * m, 512) if m > 0 else slice(0, 512)
                                nc.tensor.matmul(
                                    psq[:, ssl], ones_c[:], pat[:, :, ssl],
                                    start=first, stop=(pg == ngrp - 1),
                                    perf_mode=DR, skip_group_check=True,
                                )
                                nc.tensor.matmul(
                                    pso[:, ssl], vh_t[:, pg, :, :], pat[:, :, ssl],
                                    start=first, stop=False,
                                    perf_mode=DR, skip_group_check=True,
                                )
                                nc.tensor.matmul(
                                    pso[:, ssl], vl_t[:, pg, :, :], pat[:, :, ssl],
                                    start=False, stop=(pg == ngrp - 1),
                                    perf_mode=DR, skip_group_check=True,
                                )
                            if 2 * g >= 4 * nq:
                                # diagonal group: exp only valid suffixes,
                                # affine zeroes stale prefix + causal triangle
                                m = g - 2 * nq
                                for i in range(2):
                                    j = 2 * g + i - 4 * nq
                                    nc.scalar.activation(
                                        at[:, i, 128 * j:512],
                                        ps[:, i, 128 * j:512],
                                        mybir.ActivationFunctionType.Exp,
                                        scale=SCALE, bias=expb_t[:],
                                    )
                                    # av/rowsum read only cols >= 256m; zero
                                    # the boundary triangle (and, for i=1,
                                    # the stale 128-col prefix) of that region
                                    w0 = 256 * m
                                    nc.gpsimd.affine_select(
                                        out=at[:, i, w0:128 * (j + 1)],
                                        in_=at[:, i, w0:128 * (j + 1)],
                                        compare_op=mybir.AluOpType.is_ge,
                                        fill=0.0, base=w0 - 128 * j,
                                        pattern=[[1, 128 * (j + 1) - w0]],
                                        channel_multiplier=-1,
                                    )
                            else:
                                nc.scalar.activation(
                                    at[:].rearrange("p i l -> p (i l)"),
                                    ps[:].rearrange("p i l -> p (i l)"),
                                    mybir.ActivationFunctionType.Exp,
                                    scale=SCALE, bias=expb_t[:],
                                )
                            pending.append((at, g))
                            gctr += 1
                            while wo_queue and wo_done < gctr * nwo // total_grps:
                                wo_job(*wo_queue.pop(0))
                                wo_done += 1
                        # flush remaining groups' av
                        while pending:
                            pat, pg = pending.pop(0)
                            m = pg - 2 * nq
                            ssl = slice(256 * m, 512) if m > 0 else slice(0, 512)
                            last = not pending
                            nc.tensor.matmul(
                                psq[:, ssl], ones_c[:], pat[:, :, ssl],
                                start=(pg == 0), stop=last,
                                perf_mode=DR, skip_group_check=True,
                            )
                            nc.tensor.matmul(
                                pso[:, ssl], vh_t[:, pg, :, :], pat[:, :, ssl],
                                start=(pg == 0), stop=False,
                                perf_mode=DR, skip_group_check=True,
                            )
                            nc.tensor.matmul(
                                pso[:, ssl], vl_t[:, pg, :, :], pat[:, :, ssl],
                                start=False, stop=last,
                                perf_mode=DR, skip_group_check=True,
                            )
                        # normalization: recip (DVE), partition broadcast
                        # (gpsimd), scale+store hi/lo attn-out
                        rbs = norm_sb.tile([128, 512], F32, tag="rbs")
                        nc.vector.reciprocal(rbs[:], psq[:])
                        aot = norm_sb.tile([128, 512], BF16, tag="aot")
                        nc.vector.tensor_mul(aot[:], pso[:], rbs[:])
                        nc.gpsimd.tensor_copy(ao_h[hp][:, hi, qsl], aot[:])
                        nc.gpsimd.tensor_sub(ao_l[hp][:, hi, qsl], aot[:],
                                             ao_h[hp][:, hi, qsl])
                        pop_rope(1)
                    while wo_queue:
                        wo_job(*wo_queue.pop(0))
                    for lt in range(4 * nq, 4 * nq + 4):
                        for no in range(NQT):
                            wo_queue.append((lt, no))
                while wo_queue:
                    wo_job(*wo_queue.pop(0), tail=True)

    nc.compile()
    return nc


_ROPE_PERM = np.concatenate([np.arange(0, HD, 2), np.arange(1, HD, 2)])


def _q8(a):
    return np.clip(a, -240.0, 240.0).astype(F8)


def _prep_inputs(x, freqs_cos, freqs_sin, Wq, Wk, Wv, Wo):
    """Build the 8 per-core input maps (numpy, host-side)."""
    x = np.asarray(x, np.float32)
    cosT = np.ascontiguousarray(
        np.tile(np.asarray(freqs_cos, np.float32).T, (2, 1))).astype(BF)
    sinT = np.ascontiguousarray(
        np.tile(np.asarray(freqs_sin, np.float32).T, (2, 1))).astype(BF)
    Wq = np.asarray(Wq, np.float32)
    Wk = np.asarray(Wk, np.float32)
    Wv = np.asarray(Wv, np.float32)
    Wo = np.asarray(Wo, np.float32)

    # x layout: [p, 2c+i, l] = x[b, l, 256c+128i+p], scaled/hi-lo fp8
    def xlayout(xb):
        # xb [L, D] -> [128, 16, L]
        t = (SX * xb.T).reshape(2 * NPAIR, 128, L).transpose(1, 0, 2)
        hi = _q8(t)
        lo = _q8(t - hi.astype(np.float32))
        return np.ascontiguousarray(hi), np.ascontiguousarray(lo)

    xh_b = [xlayout(x[b]) for b in range(B)]

    def wlayout(W, ncol):
        # W [D, ncol] -> [128, NPAIR, 2, ncol] (rows 256c+128i+p)
        return np.ascontiguousarray(
            W.reshape(NPAIR, 2, 128, ncol).transpose(2, 0, 1, 3))

    in_maps = []
    for core in range(8):
        b, t = divmod(core, TP)
        wq_c = Wq[:, t * HQ * HD:(t + 1) * HQ * HD].reshape(D, HQ, HD)
        wq_c = wq_c[:, :, _ROPE_PERM].reshape(D, HQ * HD)
        wk_c = Wk[:, t * HD:(t + 1) * HD][:, _ROPE_PERM]
        wv_c = Wv[:, t * HD:(t + 1) * HD]
        wo_c = Wo[t * HQ * HD:(t + 1) * HQ * HD, :]  # [512, D]

        wvs = SW * wlayout(wv_c, HD)
        wvh_c = _q8(wvs)
        wvl_c = _q8(wvs - wvh_c.astype(np.float32))
        # wo layout: [p, hp, i, d] = Wo[128*(2hp+i)+p, d]
        wos = SWO * np.ascontiguousarray(
            wo_c.reshape(2, 2, 128, D).transpose(2, 0, 1, 3))
        woh_c = _q8(wos)
        wol_c = _q8(wos - woh_c.astype(np.float32))

        in_maps.append({
            "xh": xh_b[b][0],
            "xl": xh_b[b][1],
            "wq": _q8(SW * wlayout(wq_c, HQ * HD)),
            "wk": _q8(SW * wlayout(wk_c, HD)),
            "wvh": wvh_c,
            "wvl": wvl_c,
            "woh": woh_c,
            "wol": wol_c,
            "cosT": cosT,
            "sinT": sinT,
        })
    return in_maps


_NC_CACHE = None


def run(inputs, trace=False, trace_kwargs=None):
    global _NC_CACHE
    if _NC_CACHE is None:
        _NC_CACHE = build_nc()
    nc = _NC_CACHE
    in_maps = _prep_inputs(
        inputs["x"], inputs["freqs_cos"], inputs["freqs_sin"],
        inputs["Wq"], inputs["Wk"], inputs["Wv"], inputs["Wo"],
    )
    res = None
    last_err = None
    for attempt in range(4):
        if attempt:
            time.sleep(5.0 * attempt)  # transient device wedges clear with a pause
        try:
            res = bass_utils.run_bass_kernel_spmd(
                nc, in_maps, core_ids=list(range(8)),
                trace=trace and attempt == 0, **(trace_kwargs or {}),
            )
            break
        except ModuleNotFoundError:
            try:
                res = bass_utils.run_bass_kernel_spmd(
                    nc, in_maps, core_ids=list(range(8)), trace=False,
                )
                break
            except Exception as e:  # transient device errors: retry
                last_err = e
        except Exception as e:
            last_err = e
    if res is None:
        raise last_err
    partials = [r["out"] for r in res.results]
    out = np.empty((B, L, D), np.float32)
    for b in range(B):
        acc = partials[b * TP].astype(np.float32)
        for t in range(1, TP):
            acc = acc + partials[b * TP + t]
        out[b] = acc
    # exact host-side bias folds: +bo, and +bv @ Wo (softmax rows sum to 1).
    bo = np.asarray(inputs["bo"], np.float32)
    bv = np.asarray(inputs["bv"], np.float32)
    Wo = np.asarray(inputs["Wo"], np.float32)
    bias = bo + np.repeat(bv.reshape(KVH, HD), N_REP, axis=0).reshape(-1) @ Wo
    out += bias[None, None, :]
    return out, res


def kernel(**inputs) -> np.ndarray:
    out, _ = run(inputs, trace=False)
    return out


if __name__ == "__main__":
    pass
